# revision 22
# baseline (speedup 1.0000x reference)
"""DigitCaps dynamic-routing kernel for Trainium2 (8 NeuronCores, Bass/Tile).

Math (per routing iteration, reformulated to avoid materializing u_hat):
    u_hat[b,i,j,u] = sum_k W[i,j,u,k] * x[b,k,i]
    s[b,ju]  = sum_{ki} X[ki,b] * (c[i,j] * W[ki,ju])          (PE matmul, K=9216)
    v        = squash(s)  with the reference's quirky j-axis norm
    G[ki,ju] = sum_b X[b,ki] * v[b,ju]                         (PE matmul, K=64)
    b[i,j]   = sum_{k,u} W[ki,ju] * G[ki,ju]                   (DVE product+reduce)
    b is AllReduced (sum) over the 8 cores each iteration (batch mean).

Sharding: data-parallel over batch B=512 -> 64 rows per core; W replicated.

v2 design:
  - everything on the PE runs bf16 (fp32 LOW_HIGH matmuls were ~5x slower);
    validated end-to-end L2 err ~3e-3 vs the 2e-2 gate.
  - host pre-casts x/W to bf16 and pre-builds both layouts (natural + transposed
    x, (j,u,k) + (k,j,u) W) so the chip does zero load-phase transposes/casts;
    the load phase is pure DMA (~3.8MB total).
  - softmax uses unnormalized exp: wp = exp(b/B) * W starts per-t2-chunk as
    soon as the AllReduce lands; the 1/Z normalizer is folded into a single
    post-matmul multiply (Z from an accumulating ones-matmul).
  - squash sqrt is a DVE Newton rsqrt (quake seed + 2 iterations), so the ACT
    exp table never reloads (table switches cost ~1.5us each).
  - b-update: G PSUM banks are read directly by a DVE/GPSIMD product into a
    (j,u,k)-ordered bf16 tensor, then one DVE X-reduce per t2 -> b_part.
  - ONE AllReduce per iteration (bf16 payload), plus a tiny warm-up AllReduce
    at t=0 to absorb the ~26us cold-start of the collective stack.
  - dummy warm-up matmuls during the load phase keep the PE HAM un-throttled.
"""

import sys

sys.path.insert(0, "/opt/trn_rl_repo")

from contextlib import ExitStack

import numpy as np

B = 512
NCORES = 8
BL = B // NCORES  # 64 local batch rows
K = 8             # in_units (primary capsule dim)
IC = 1152         # in_channels (number of primary capsules)
J = 10            # num_units (output capsules)
U = 16            # unit_size
JU = J * U        # 160
NT = IC // 128    # 9 i-chunks of 128
NKT = K * NT      # 72 ki-chunks of 128
BETA = 1.45
NUM_ROUTING = 3

_CACHE = {}


def _build_nc():
    import concourse.bass as bass
    import concourse.tile as tile
    from concourse import bacc, mybir
    from concourse.masks import make_identity

    f32 = mybir.dt.float32
    bf16 = mybir.dt.bfloat16
    i32 = mybir.dt.int32
    Alu = mybir.AluOpType
    Act = mybir.ActivationFunctionType
    Ax = mybir.AxisListType

    nc = bacc.Bacc("TRN2", target_bir_lowering=False, debug=False,
                   num_devices=NCORES)

    # host-prepped bf16 inputs (see _prep below); x natural layout and W
    # (j,u,k) order are rebuilt ON-CHIP (PE transposes / DVE copies) to
    # halve the DMA footprint -- DMA runs ~6.5GB/s per engine here.
    xs1 = nc.dram_tensor("xs1", [128, NKT, BL], bf16, kind="ExternalInput").ap()
    wk = nc.dram_tensor("wk", [128, NT * JU * K], bf16, kind="ExternalInput").ap()
    wn = nc.dram_tensor("wn", [128, NT * JU * K], bf16, kind="ExternalInput").ap()
    out = nc.dram_tensor("out", [BL, JU], f32, kind="ExternalOutput").ap()

    with tile.TileContext(nc) as tc, ExitStack() as ctx:
        consts = ctx.enter_context(tc.tile_pool(name="consts", bufs=1))
        small = ctx.enter_context(tc.tile_pool(name="small", bufs=2))
        scratch = ctx.enter_context(tc.tile_pool(name="scratch", bufs=8))
        psum = ctx.enter_context(tc.tile_pool(name="psum", bufs=1, space="PSUM"))
        dram = ctx.enter_context(tc.tile_pool(name="dram", bufs=1, space="DRAM"))

        # ---- persistent SBUF tensors ----
        x2b = consts.tile([BL, K * IC], bf16)        # x[b, (k i)] (G stationary)
        x1b = consts.tile([128, NKT, BL], bf16)      # x^T per ki-chunk (s stationary)
        w_natb = consts.tile([128, NT, JU * K], bf16)  # W[(i),(j,u,k)]
        w_kju = consts.tile([128, NT, K * JU], bf16)   # W[(i),(k,j,u)]
        wp = consts.tile([128, NT, K * JU], bf16)      # exp-scaled W (iters>0)
        ones = consts.tile([128, 128], bf16)         # Z broadcast matmul lhsT

        # one PSUM tensor = all 8 banks; everything slices into it
        pall = psum.tile([128, K, 512], f32)

        nc.vector.memset(ones, 1.0)

        # ---- ACT exp-table preload (the only ACT table this kernel uses) ----
        etp = consts.tile([BL, 1], f32)
        nc.vector.memset(etp, 0.0)
        nc.scalar.activation(etp, etp, Act.Exp, scale=1.0)

        # ---- loads: ~14 pieces so every DMA engine streams one in parallel
        # (per-engine rate is only ~6.5GB/s; per-engine queue is serial).
        # Triggers split across the two HWDGE queues (sync + scalar). ----
        wk_flat = w_kju.rearrange("p t f -> p (t f)")
        wn_flat = w_natb.rearrange("p t f -> p (t f)")
        for k in range(K):
            nc.sync.dma_start(out=x1b[:, k * NT:(k + 1) * NT, :],
                              in_=xs1[:, k * NT:(k + 1) * NT, :])
        for c in range(6):
            nc.scalar.dma_start(out=wk_flat[:, c * 1920:(c + 1) * 1920],
                                in_=wk[:, c * 1920:(c + 1) * 1920])
        # second wave: w_natb is only needed by the it0 G phase (~12us after
        # the first wave completes); alternate queues for parallel issue
        for c in range(8):
            eng = nc.sync if c % 2 == 0 else nc.scalar
            eng.dma_start(out=wn_flat[:, c * 1440:(c + 1) * 1440],
                          in_=wn[:, c * 1440:(c + 1) * 1440])

        # ---- rebuild x2b = x1b^T on the PE (bf16 transposes, PSUM bitcast,
        # ACT/DVE evacuation). Doubles as the HAM warm-up: the PE stays busy
        # through the load phase. First half now; second half is emitted
        # after the it0 s-matmuls (x2b isn't needed until the G phase). ----
        ident = consts.tile([128, 128], bf16)
        make_identity(nc, ident)
        pbf = pall.bitcast(bf16)                    # [128, K, 1024] bf16 view

        def transpose_chunk(t, slot):
            k, t2 = divmod(t, NT)
            ps = pbf[:BL, slot, :128]               # [64, 128] bf16
            nc.tensor.transpose(ps, x1b[:, t, :], ident)
            dst = x2b[:, k * IC + t2 * 128:k * IC + t2 * 128 + 128]
            nc.scalar.copy(dst, ps)

        for t in range(NKT // 2):
            transpose_chunk(t, t % 8)

        # 4D views for the b-update product
        w4 = w_natb.rearrange("p t (j u k) -> p t j u k", j=J, u=U)

        bf_tiles = {}
        for it in range(NUM_ROUTING):
            # ---- wp = exp(b_sum/B) * w_kju (iters>0). Iteration 0 has
            # uniform c = 1/IC folded into the squash scales, so the matmul
            # rhs is just w_kju directly. ----
            if it > 0:
                bf_t = bf_tiles[it - 1]
                expb = small.tile([128, NT, J], bf16, name=f"expb{it}")
                for t2 in range(NT):
                    nc.scalar.activation(expb[:, t2, :], bf_t[:, t2, :],
                                         Act.Exp, scale=1.0 / B)
                    nc.vector.tensor_mul(
                        wp[:, t2, :].rearrange("p (k j u) -> p k j u",
                                               k=K, j=J),
                        w_kju[:, t2, :].rearrange("p (k j u) -> p k j u",
                                                  k=K, j=J),
                        expb[:, t2, :].unsqueeze(1).unsqueeze(-1)
                        .broadcast_to([128, K, J, U]))
                rhs_src = wp
            else:
                rhs_src = w_kju

            # ---- s = X1^T @ wp : accumulate 72 chunks into PSUM bank 0 ----
            sp = pall[:BL, 0, :JU]
            for t2 in range(NT):
                for k in range(K):
                    t = k * NT + t2
                    first = (t2 == 0 and k == 0)
                    last = (t2 == NT - 1 and k == K - 1)
                    nc.tensor.matmul(sp, x1b[:, t, :],
                                     rhs_src[:, t2, k * JU:(k + 1) * JU],
                                     start=first, stop=last)

            if it == 0:
                # second half of the x2b transposes (banks 1-7: bank 0 holds
                # the s accumulation); needed only by the G phase below
                for t in range(NKT // 2, NKT):
                    transpose_chunk(t, 1 + t % 7)

            # ---- Z[j] = sum_i exp(b[i,j]) via accumulating ones-matmul ----
            if it > 0:
                zp = pall[:, 1, :J]
                for t2 in range(NT):
                    nc.tensor.matmul(zp, ones, expb[:, t2, :],
                                     start=(t2 == 0), stop=(t2 == NT - 1))
                zinv = small.tile([BL, J], f32, name=f"zinv{it}")
                nc.vector.reciprocal(zinv, zp[:BL, :])
                # s_norm = s * (1/Z_j), also evacuates PSUM
                s_sb = small.tile([BL, JU], f32, name=f"s_sb{it}")
                nc.vector.tensor_mul(
                    s_sb.rearrange("b (j u) -> b j u", j=J),
                    sp.rearrange("b (j u) -> b j u", j=J),
                    zinv.unsqueeze(-1).broadcast_to([BL, J, U]))
            else:
                s_sb = small.tile([BL, JU], f32, name=f"s_sb{it}")
                nc.vector.tensor_copy(s_sb, sp)

            # ---- squash (reference quirk: norm over the j axis per (b,u)).
            # All on DVE; sqrt via quake-seed Newton rsqrt (no ACT tables). ----
            ssq = small.tile([BL, JU], f32, name=f"ssq{it}")
            nc.vector.tensor_mul(ssq, s_sb, s_sb)
            msq = small.tile([BL, U], f32, name=f"msq{it}")
            nc.vector.tensor_reduce(
                msq, ssq.rearrange("b (j u) -> b u j", j=J),
                axis=Ax.X, op=Alu.add)
            # iteration 0: s here is actually IC*s, so scale msq by 1/IC^2
            # and fold 1/IC into the final v multiply
            sc2 = 1.0 / (IC * IC) if it == 0 else 1.0
            scv = 1.0 / (IC * IC) if it == 0 else 1.0
            # y ~= rsqrt(msq): quake seed + 2 Newton iterations
            ti = small.tile([BL, U], i32, name=f"ti{it}")
            nc.vector.tensor_scalar(ti, msq.bitcast(i32), 1, 0,
                                    op0=Alu.arith_shift_right,
                                    op1=Alu.logical_shift_left)
            y0i = small.tile([BL, U], i32, name=f"y0i{it}")
            nc.vector.tensor_scalar(y0i, ti, 0x5f3759df, -1,
                                    op0=Alu.subtract, op1=Alu.mult)
            y0 = y0i.bitcast(f32)
            half = small.tile([BL, U], f32, name=f"half{it}")
            nc.vector.tensor_scalar(half, msq, 0.5, 0.0,
                                    op0=Alu.mult, op1=Alu.add)
            yc = y0
            for n in range(2):
                t_a = small.tile([BL, U], f32, name=f"na{it}_{n}")
                nc.vector.tensor_mul(t_a, yc, yc)
                nc.vector.tensor_mul(t_a, t_a, half)
                nc.vector.tensor_scalar(t_a, t_a, -1.0, 1.5,
                                        op0=Alu.mult, op1=Alu.add)
                t_b = small.tile([BL, U], f32, name=f"nb{it}_{n}")
                nc.vector.tensor_mul(t_b, yc, t_a)
                yc = t_b
            # f = msq*y * 1/(beta + msq*sc2) (scaled for it0)
            tpb = small.tile([BL, U], f32, name=f"tpb{it}")
            nc.vector.tensor_scalar(tpb, msq, sc2, BETA,
                                    op0=Alu.mult, op1=Alu.add)
            rin = small.tile([BL, U], f32, name=f"rin{it}")
            nc.vector.reciprocal(rin, tpb)
            fv = small.tile([BL, U], f32, name=f"fv{it}")
            nc.vector.tensor_mul(fv, msq, yc)
            nc.vector.tensor_mul(fv, fv, rin)
            v = small.tile([BL, JU], f32, name=f"v{it}")
            nc.vector.scalar_tensor_tensor(
                out=v.rearrange("b (j u) -> b j u", j=J),
                in0=s_sb.rearrange("b (j u) -> b j u", j=J),
                scalar=scv,
                in1=fv.unsqueeze(1).broadcast_to([BL, J, U]),
                op0=Alu.mult, op1=Alu.mult)

            if it == NUM_ROUTING - 1:
                nc.sync.dma_start(out=out, in_=v)
                continue
            vb = small.tile([BL, JU], bf16, name=f"vb{it}")
            nc.vector.tensor_copy(vb, v)

            # ---- G = X2^T-chunks @ v per t2; banks ping-pong in halves
            # (even t2 -> banks 0-3, odd -> 4-7; k packed 2-per-bank).
            # b_part[:, t2, j] = sum_{u,k} W * G via product + X-reduce. ----
            b_part = small.tile([128, NT, J], f32, name=f"bpart{it}")
            b_bf = small.tile([128, NT, J], bf16, name=f"bbf{it}")
            for t2 in range(NT):
                b0 = 0 if t2 % 2 == 0 else 4
                for k in range(K):
                    bank = b0 + k // 2
                    kk = k % 2
                    nc.tensor.matmul(
                        pall[:, bank, kk * JU:(kk + 1) * JU],
                        x2b[:, (k * IC + t2 * 128):(k * IC + t2 * 128) + 128],
                        vb, start=True, stop=True)
                # product P[(j,u,k)] = W * G, G read straight out of PSUM
                # via a 4D AP [j, u, bank, kk]
                # The PSUM bank layout (bank, kk, ju) read in linear order IS
                # (k, j, u) order -> contiguous evacuation, contiguous product
                # against w_kju (2x DVE mode), and the k-reduction becomes
                # contiguous TT-add folds (2x) instead of a 1x strided reduce.
                HF = JU * K // 2
                g5 = scratch.tile([128, JU * K], bf16, name="g5", bufs=3)
                nc.scalar.copy(g5[:, :HF].rearrange("p (b f) -> p b f", b=2),
                               pall[:, b0:b0 + 2, :2 * JU])
                nc.scalar.copy(g5[:, HF:].rearrange("p (b f) -> p b f", b=2),
                               pall[:, b0 + 2:b0 + 4, :2 * JU])
                prod = scratch.tile([128, JU * K], bf16, name="prod", bufs=3)
                nc.vector.tensor_mul(prod, w_kju[:, t2, :], g5)
                p3 = prod.rearrange("p (k f) -> p k f", k=K)
                f4 = scratch.tile([128, 4, JU], bf16, name="f4", bufs=3)
                nc.gpsimd.tensor_add(f4, p3[:, :4], p3[:, 4:])
                f2 = scratch.tile([128, 2, JU], bf16, name="f2", bufs=3)
                nc.vector.tensor_add(f2, f4[:, :2], f4[:, 2:])
                f1 = scratch.tile([128, JU], bf16, name="f1", bufs=3)
                nc.vector.tensor_add(f1, f2[:, 0], f2[:, 1])
                nc.vector.tensor_reduce(
                    b_part[:, t2, :],
                    f1.rearrange("p (j u) -> p j u", j=J),
                    axis=Ax.X, op=Alu.add)
                nc.scalar.copy(b_bf[:, t2, :], b_part[:, t2, :])

            # ---- AllReduce b over the 8 cores (bf16 payload) ----
            cc_in = dram.tile([IC, J], bf16, name=f"ccin{it}")
            cc_out = dram.tile([IC, J], bf16, name=f"ccout{it}",
                               addr_space="Shared")
            nc.sync.dma_start(
                out=cc_in.rearrange("(t p) j -> p t j", p=128),
                in_=b_bf)
            nc.gpsimd.collective_compute(
                "AllReduce", Alu.add,
                replica_groups=[list(range(NCORES))],
                ins=[cc_in[:, :]], outs=[cc_out[:, :]])
            bf_t = small.tile([128, NT, J], bf16, name=f"bf{it}")
            nc.sync.dma_start(
                out=bf_t, in_=cc_out.rearrange("(t p) j -> p t j", p=128))
            bf_tiles[it] = bf_t

    nc.compile()
    return nc


def _prep(x, W):
    """Host-side prep: bf16 cast + device layouts for x and W."""
    import ml_dtypes

    bf16 = ml_dtypes.bfloat16
    x = np.asarray(x, dtype=np.float32)
    W = np.asarray(W, dtype=np.float32)
    xb = x.astype(bf16)                      # (B, K, IC)
    # W natural (j,u,k): [p, (t2, j u k)]
    wn = np.ascontiguousarray(
        W.reshape(NT, 128, J * U * K).transpose(1, 0, 2)
        .reshape(128, NT * J * U * K).astype(bf16))
    # W (k,j,u): [p, (t2, k j u)]
    wk = np.ascontiguousarray(
        W.reshape(NT, 128, J, U, K).transpose(1, 0, 4, 2, 3)
        .reshape(128, NT * K * J * U).astype(bf16))
    in_maps = []
    for c in range(NCORES):
        rows = xb[c * BL:(c + 1) * BL]       # (BL, K, IC)
        xs1 = np.ascontiguousarray(
            rows.reshape(BL, K, NT, 128).transpose(3, 1, 2, 0)
            .reshape(128, NKT, BL))
        in_maps.append({
            "xs1": xs1,
            "wn": wn,
            "wk": wk,
        })
    return in_maps


def _run(x, W, trace=False, **kw):
    from concourse import bass_utils

    nc = _get_nc()
    in_maps = _prep(x, W)
    res = bass_utils.run_bass_kernel_spmd(
        nc, in_maps, core_ids=list(range(NCORES)), trace=trace, **kw)
    outs = [res.results[c]["out"] for c in range(NCORES)]
    full = np.concatenate(outs, axis=0).reshape(B, J, 4, U // 4)
    return full, res


def _get_nc():
    if "nc" not in _CACHE:
        _CACHE["nc"] = _build_nc()
    return _CACHE["nc"]


def kernel(x, W):
    full, _ = _run(x, W, trace=False)
    return full


# revision 27
# speedup vs baseline: 1.1957x; 1.1957x over previous
"""DigitCaps dynamic-routing kernel for Trainium2 (8 NeuronCores, Bass/Tile).

Math (per routing iteration, reformulated to avoid materializing u_hat):
    u_hat[b,i,j,u] = sum_k W[i,j,u,k] * x[b,k,i]
    s[b,ju]  = sum_{ki} X[ki,b] * (c[i,j] * W[ki,ju])          (PE matmul, K=9216)
    v        = squash(s)  with the reference's quirky j-axis norm
    G[ki,ju] = sum_b X[b,ki] * v[b,ju]                         (PE matmul, K=64)
    b[i,j]   = sum_{k,u} W[ki,ju] * G[ki,ju]                   (DVE product+reduce)
    b is AllReduced (sum) over the 8 cores each iteration (batch mean).

Sharding: data-parallel over batch B=512 -> 64 rows per core; W replicated.

v2 design:
  - everything on the PE runs bf16 (fp32 LOW_HIGH matmuls were ~5x slower);
    validated end-to-end L2 err ~3e-3 vs the 2e-2 gate.
  - host pre-casts x/W to bf16 and pre-builds both layouts (natural + transposed
    x, (j,u,k) + (k,j,u) W) so the chip does zero load-phase transposes/casts;
    the load phase is pure DMA (~3.8MB total).
  - softmax uses unnormalized exp: wp = exp(b/B) * W starts per-t2-chunk as
    soon as the AllReduce lands; the 1/Z normalizer is folded into a single
    post-matmul multiply (Z from an accumulating ones-matmul).
  - squash sqrt is a DVE Newton rsqrt (quake seed + 2 iterations), so the ACT
    exp table never reloads (table switches cost ~1.5us each).
  - b-update: G PSUM banks are read directly by a DVE/GPSIMD product into a
    (j,u,k)-ordered bf16 tensor, then one DVE X-reduce per t2 -> b_part.
  - ONE AllReduce per iteration (bf16 payload), plus a tiny warm-up AllReduce
    at t=0 to absorb the ~26us cold-start of the collective stack.
  - dummy warm-up matmuls during the load phase keep the PE HAM un-throttled.
"""

import sys

sys.path.insert(0, "/opt/trn_rl_repo")

from contextlib import ExitStack

import numpy as np

B = 512
NCORES = 8
BL = B // NCORES  # 64 local batch rows
K = 8             # in_units (primary capsule dim)
IC = 1152         # in_channels (number of primary capsules)
J = 10            # num_units (output capsules)
U = 16            # unit_size
JU = J * U        # 160
NT = IC // 128    # 9 i-chunks of 128
NKT = K * NT      # 72 ki-chunks of 128
BETA = 1.45
NUM_ROUTING = 3

_CACHE = {}


def _build_nc():
    import concourse.bass as bass
    import concourse.tile as tile
    from concourse import bacc, mybir
    from concourse.masks import make_identity

    f32 = mybir.dt.float32
    bf16 = mybir.dt.bfloat16
    i32 = mybir.dt.int32
    Alu = mybir.AluOpType
    Act = mybir.ActivationFunctionType
    Ax = mybir.AxisListType

    nc = bacc.Bacc("TRN2", target_bir_lowering=False, debug=False,
                   num_devices=NCORES)

    # host-prepped bf16 inputs (see _prep below); x natural layout and W
    # (j,u,k) order are rebuilt ON-CHIP (PE transposes / DVE copies) to
    # halve the DMA footprint -- DMA runs ~6.5GB/s per engine here.
    xs1 = nc.dram_tensor("xs1", [128, NKT, BL], bf16, kind="ExternalInput").ap()
    wk = nc.dram_tensor("wk", [128, NT * JU * K], bf16, kind="ExternalInput").ap()
    wn = nc.dram_tensor("wn", [128, NT * JU * K], bf16, kind="ExternalInput").ap()
    out = nc.dram_tensor("out", [BL, JU], f32, kind="ExternalOutput").ap()

    with tile.TileContext(nc) as tc, ExitStack() as ctx:
        consts = ctx.enter_context(tc.tile_pool(name="consts", bufs=1))
        small = ctx.enter_context(tc.tile_pool(name="small", bufs=2))
        scratch = ctx.enter_context(tc.tile_pool(name="scratch", bufs=8))
        psum = ctx.enter_context(tc.tile_pool(name="psum", bufs=1, space="PSUM"))
        dram = ctx.enter_context(tc.tile_pool(name="dram", bufs=1, space="DRAM"))

        # ---- persistent SBUF tensors ----
        x2b = consts.tile([BL, K * IC], bf16)        # x[b, (k i)] (G stationary)
        x1b = consts.tile([128, NKT, BL], bf16)      # x^T per ki-chunk (s stationary)
        w_natb = consts.tile([128, NT, JU * K], bf16)  # W[(i),(j,u,k)]
        w_kju = consts.tile([128, NT, K * JU], bf16)   # W[(i),(k,j,u)]
        wp = consts.tile([128, NT, K * JU], bf16)      # exp-scaled W (iters>0)
        ones = consts.tile([128, 128], bf16)         # Z broadcast matmul lhsT

        # one PSUM tensor = all 8 banks; everything slices into it
        pall = psum.tile([128, K, 512], f32)

        nc.vector.memset(ones, 1.0)

        # ---- ACT exp-table preload (the only ACT table this kernel uses) ----
        etp = consts.tile([BL, 1], f32)
        nc.vector.memset(etp, 0.0)
        nc.scalar.activation(etp, etp, Act.Exp, scale=1.0)

        # ---- warm-up AllReduce: the first collective after the ncfw barrier
        # pays a ~20us cold cost; firing a tiny one early absorbs it during
        # the it0 compute so the real b AllReduce runs warm. ----
        warm_in = dram.tile([BL, 1], f32, name="warm_in")
        warm_out = dram.tile([BL, 1], f32, name="warm_out",
                             addr_space="Shared")
        nc.sync.dma_start(out=warm_in, in_=etp)
        nc.gpsimd.collective_compute(
            "AllReduce", Alu.add,
            replica_groups=[list(range(NCORES))],
            ins=[warm_in[:, :]], outs=[warm_out[:, :]])

        # ---- loads: ~14 pieces so every DMA engine streams one in parallel
        # (per-engine rate is only ~6.5GB/s; per-engine queue is serial).
        # Triggers split across the two HWDGE queues (sync + scalar). ----
        wk_flat = w_kju.rearrange("p t f -> p (t f)")
        wn_flat = w_natb.rearrange("p t f -> p (t f)")
        for k in range(K):
            nc.sync.dma_start(out=x1b[:, k * NT:(k + 1) * NT, :],
                              in_=xs1[:, k * NT:(k + 1) * NT, :])
        for c in range(6):
            nc.scalar.dma_start(out=wk_flat[:, c * 1920:(c + 1) * 1920],
                                in_=wk[:, c * 1920:(c + 1) * 1920])
        # second wave: w_natb is only needed by the it0 G phase (~12us after
        # the first wave completes); alternate queues for parallel issue
        for c in range(8):
            eng = nc.sync if c % 2 == 0 else nc.scalar
            eng.dma_start(out=wn_flat[:, c * 1440:(c + 1) * 1440],
                          in_=wn[:, c * 1440:(c + 1) * 1440])

        # ---- rebuild x2b = x1b^T on the PE (bf16 transposes, PSUM bitcast,
        # ACT/DVE evacuation). Doubles as the HAM warm-up: the PE stays busy
        # through the load phase. First half now; second half is emitted
        # after the it0 s-matmuls (x2b isn't needed until the G phase). ----
        ident = consts.tile([128, 128], bf16)
        make_identity(nc, ident)
        pbf = pall.bitcast(bf16)                    # [128, K, 1024] bf16 view

        def transpose_chunk(t, slot):
            k, t2 = divmod(t, NT)
            ps = pbf[:BL, slot, :128]               # [64, 128] bf16
            nc.tensor.transpose(ps, x1b[:, t, :], ident)
            dst = x2b[:, k * IC + t2 * 128:k * IC + t2 * 128 + 128]
            nc.scalar.copy(dst, ps)

        for t in range(NKT // 2):
            transpose_chunk(t, t % 8)

        # 4D views for the b-update product
        w4 = w_natb.rearrange("p t (j u k) -> p t j u k", j=J, u=U)

        bf_tiles = {}
        for it in range(NUM_ROUTING):
            # ---- wp = exp(b_sum/B) * w_kju (iters>0). Iteration 0 has
            # uniform c = 1/IC folded into the squash scales, so the matmul
            # rhs is just w_kju directly. ----
            if it > 0:
                bf_t = bf_tiles[it - 1]
                expb = small.tile([128, NT, J], bf16, name=f"expb{it}")
                nc.scalar.activation(
                    expb.rearrange("p t j -> p (t j)"),
                    bf_t.rearrange("p t j -> p (t j)"), Act.Exp, scale=1.0 / B)
                for t2 in range(NT):
                    nc.vector.tensor_mul(
                        wp[:, t2, :].rearrange("p (k j u) -> p k j u",
                                               k=K, j=J),
                        w_kju[:, t2, :].rearrange("p (k j u) -> p k j u",
                                                  k=K, j=J),
                        expb[:, t2, :].unsqueeze(1).unsqueeze(-1)
                        .broadcast_to([128, K, J, U]))
                rhs_src = wp
            else:
                rhs_src = w_kju

            # ---- s = X1^T @ wp : accumulate 72 chunks into PSUM bank 0 ----
            sp = pall[:BL, 0, :JU]
            for t2 in range(NT):
                for k in range(K):
                    t = k * NT + t2
                    first = (t2 == 0 and k == 0)
                    last = (t2 == NT - 1 and k == K - 1)
                    nc.tensor.matmul(sp, x1b[:, t, :],
                                     rhs_src[:, t2, k * JU:(k + 1) * JU],
                                     start=first, stop=last)

            if it == 0:
                # second half of the x2b transposes (banks 1-7: bank 0 holds
                # the s accumulation); needed only by the G phase below
                for t in range(NKT // 2, NKT):
                    transpose_chunk(t, 1 + t % 7)

            # ---- Z[j] = sum_i exp(b[i,j]) via accumulating ones-matmul ----
            if it > 0:
                zp = pall[:, 1, :J]
                for t2 in range(NT):
                    nc.tensor.matmul(zp, ones, expb[:, t2, :],
                                     start=(t2 == 0), stop=(t2 == NT - 1))
                zinv = small.tile([BL, J], f32, name=f"zinv{it}")
                nc.vector.reciprocal(zinv, zp[:BL, :])
                # s_norm = s * (1/Z_j), also evacuates PSUM
                s_sb = small.tile([BL, JU], f32, name=f"s_sb{it}")
                nc.vector.tensor_mul(
                    s_sb.rearrange("b (j u) -> b j u", j=J),
                    sp.rearrange("b (j u) -> b j u", j=J),
                    zinv.unsqueeze(-1).broadcast_to([BL, J, U]))
            else:
                s_sb = small.tile([BL, JU], f32, name=f"s_sb{it}")
                nc.vector.tensor_copy(s_sb, sp)

            # ---- squash (reference quirk: norm over the j axis per (b,u)).
            # All on DVE; sqrt via quake-seed Newton rsqrt (no ACT tables). ----
            ssq = small.tile([BL, JU], f32, name=f"ssq{it}")
            nc.vector.tensor_mul(ssq, s_sb, s_sb)
            msq = small.tile([BL, U], f32, name=f"msq{it}")
            nc.vector.tensor_reduce(
                msq, ssq.rearrange("b (j u) -> b u j", j=J),
                axis=Ax.X, op=Alu.add)
            # iteration 0: s here is actually IC*s, so scale msq by 1/IC^2
            # and fold 1/IC into the final v multiply
            sc2 = 1.0 / (IC * IC) if it == 0 else 1.0
            scv = 1.0 / (IC * IC) if it == 0 else 1.0
            # y ~= rsqrt(msq): quake seed + 2 Newton iterations
            ti = small.tile([BL, U], i32, name=f"ti{it}")
            nc.vector.tensor_scalar(ti, msq.bitcast(i32), 1, 0,
                                    op0=Alu.arith_shift_right,
                                    op1=Alu.logical_shift_left)
            y0i = small.tile([BL, U], i32, name=f"y0i{it}")
            nc.vector.tensor_scalar(y0i, ti, 0x5f3759df, -1,
                                    op0=Alu.subtract, op1=Alu.mult)
            y0 = y0i.bitcast(f32)
            half = small.tile([BL, U], f32, name=f"half{it}")
            nc.vector.tensor_scalar(half, msq, 0.5, 0.0,
                                    op0=Alu.mult, op1=Alu.add)
            yc = y0
            for n in range(2):
                t_a = small.tile([BL, U], f32, name=f"na{it}_{n}")
                nc.vector.tensor_mul(t_a, yc, yc)
                nc.vector.tensor_mul(t_a, t_a, half)
                nc.vector.tensor_scalar(t_a, t_a, -1.0, 1.5,
                                        op0=Alu.mult, op1=Alu.add)
                t_b = small.tile([BL, U], f32, name=f"nb{it}_{n}")
                nc.vector.tensor_mul(t_b, yc, t_a)
                yc = t_b
            # f = msq*y * 1/(beta + msq*sc2) (scaled for it0)
            tpb = small.tile([BL, U], f32, name=f"tpb{it}")
            nc.vector.tensor_scalar(tpb, msq, sc2, BETA,
                                    op0=Alu.mult, op1=Alu.add)
            rin = small.tile([BL, U], f32, name=f"rin{it}")
            nc.vector.reciprocal(rin, tpb)
            fv = small.tile([BL, U], f32, name=f"fv{it}")
            nc.vector.tensor_mul(fv, msq, yc)
            nc.vector.tensor_mul(fv, fv, rin)
            v = small.tile([BL, JU], f32, name=f"v{it}")
            nc.vector.scalar_tensor_tensor(
                out=v.rearrange("b (j u) -> b j u", j=J),
                in0=s_sb.rearrange("b (j u) -> b j u", j=J),
                scalar=scv,
                in1=fv.unsqueeze(1).broadcast_to([BL, J, U]),
                op0=Alu.mult, op1=Alu.mult)

            if it == NUM_ROUTING - 1:
                nc.sync.dma_start(out=out, in_=v)
                continue
            vb = small.tile([BL, JU], bf16, name=f"vb{it}")
            nc.vector.tensor_copy(vb, v)

            # ---- G = X2^T-chunks @ v per t2; banks ping-pong in halves
            # (even t2 -> banks 0-3, odd -> 4-7; k packed 2-per-bank).
            # b_part[:, t2, j] = sum_{u,k} W * G via product + X-reduce. ----
            b_part = small.tile([128, NT, J], f32, name=f"bpart{it}")
            b_bf = small.tile([128, NT, J], bf16, name=f"bbf{it}")
            for t2 in range(NT):
                b0 = 0 if t2 % 2 == 0 else 4
                for k in range(K):
                    bank = b0 + k // 2
                    kk = k % 2
                    nc.tensor.matmul(
                        pall[:, bank, kk * JU:(kk + 1) * JU],
                        x2b[:, (k * IC + t2 * 128):(k * IC + t2 * 128) + 128],
                        vb, start=True, stop=True)
                # product P[(j,u,k)] = W * G, G read straight out of PSUM
                # via a 4D AP [j, u, bank, kk]
                # The PSUM bank layout (bank, kk, ju) read in linear order IS
                # (k, j, u) order -> contiguous evacuation, contiguous product
                # against w_kju (2x DVE mode), and the k-reduction becomes
                # contiguous TT-add folds (2x) instead of a 1x strided reduce.
                HF = JU * K // 2
                g5 = scratch.tile([128, JU * K], bf16, name="g5", bufs=3)
                nc.scalar.copy(g5[:, :HF].rearrange("p (b f) -> p b f", b=2),
                               pall[:, b0:b0 + 2, :2 * JU])
                nc.scalar.copy(g5[:, HF:].rearrange("p (b f) -> p b f", b=2),
                               pall[:, b0 + 2:b0 + 4, :2 * JU])
                prod = scratch.tile([128, JU * K], bf16, name="prod", bufs=3)
                nc.vector.tensor_mul(prod, w_kju[:, t2, :], g5)
                p3 = prod.rearrange("p (k f) -> p k f", k=K)
                f4 = scratch.tile([128, 4, JU], bf16, name="f4", bufs=3)
                nc.vector.tensor_add(f4, p3[:, :4], p3[:, 4:])
                f2 = scratch.tile([128, 2, JU], bf16, name="f2", bufs=3)
                nc.gpsimd.tensor_add(f2, f4[:, :2], f4[:, 2:])
                f1 = scratch.tile([128, JU], bf16, name="f1", bufs=3)
                nc.vector.tensor_add(f1, f2[:, 0], f2[:, 1])
                nc.vector.tensor_reduce(
                    b_part[:, t2, :],
                    f1.rearrange("p (j u) -> p j u", j=J),
                    axis=Ax.X, op=Alu.add)
                nc.scalar.copy(b_bf[:, t2, :], b_part[:, t2, :])

            # ---- AllReduce b over the 8 cores (bf16 payload). The staging
            # DMAs are split across queues: a single 23KB DMA adds ~4us of
            # exposed latency right on the critical path. ----
            cc_in = dram.tile([IC, J], bf16, name=f"ccin{it}")
            cc_out = dram.tile([IC, J], bf16, name=f"ccout{it}",
                               addr_space="Shared")
            cc_in_v = cc_in.rearrange("(t p) j -> p t j", p=128)
            for h, eng in ((0, nc.sync), (1, nc.scalar)):
                hs = slice(h * 64, (h + 1) * 64)
                eng.dma_start(out=cc_in_v[hs], in_=b_bf[hs])
            nc.gpsimd.collective_compute(
                "AllReduce", Alu.add,
                replica_groups=[list(range(NCORES))],
                ins=[cc_in[:, :]], outs=[cc_out[:, :]])
            bf_t = small.tile([128, NT, J], bf16, name=f"bf{it}")
            cc_out_v = cc_out.rearrange("(t p) j -> p t j", p=128)
            for h, eng in ((0, nc.sync), (1, nc.scalar)):
                hs = slice(h * 64, (h + 1) * 64)
                eng.dma_start(out=bf_t[hs], in_=cc_out_v[hs])
            bf_tiles[it] = bf_t

    nc.compile()
    return nc


def _prep(x, W):
    """Host-side prep: bf16 cast + device layouts for x and W."""
    import ml_dtypes

    bf16 = ml_dtypes.bfloat16
    x = np.asarray(x, dtype=np.float32)
    W = np.asarray(W, dtype=np.float32)
    xb = x.astype(bf16)                      # (B, K, IC)
    # W natural (j,u,k): [p, (t2, j u k)]
    wn = np.ascontiguousarray(
        W.reshape(NT, 128, J * U * K).transpose(1, 0, 2)
        .reshape(128, NT * J * U * K).astype(bf16))
    # W (k,j,u): [p, (t2, k j u)]
    wk = np.ascontiguousarray(
        W.reshape(NT, 128, J, U, K).transpose(1, 0, 4, 2, 3)
        .reshape(128, NT * K * J * U).astype(bf16))
    in_maps = []
    for c in range(NCORES):
        rows = xb[c * BL:(c + 1) * BL]       # (BL, K, IC)
        xs1 = np.ascontiguousarray(
            rows.reshape(BL, K, NT, 128).transpose(3, 1, 2, 0)
            .reshape(128, NKT, BL))
        in_maps.append({
            "xs1": xs1,
            "wn": wn,
            "wk": wk,
        })
    return in_maps


def _run(x, W, trace=False, **kw):
    from concourse import bass_utils

    nc = _get_nc()
    in_maps = _prep(x, W)
    res = bass_utils.run_bass_kernel_spmd(
        nc, in_maps, core_ids=list(range(NCORES)), trace=trace, **kw)
    outs = [res.results[c]["out"] for c in range(NCORES)]
    full = np.concatenate(outs, axis=0).reshape(B, J, 4, U // 4)
    return full, res


def _get_nc():
    if "nc" not in _CACHE:
        _CACHE["nc"] = _build_nc()
    return _CACHE["nc"]


def kernel(x, W):
    full, _ = _run(x, W, trace=False)
    return full
